# revision 27
# baseline (speedup 1.0000x reference)
"""Multi-head attention (B=2, N=2048, D=1024, H=16, HD=64) on 8 trn2 NeuronCores.

Sharding: data-parallel over batch (2) x tensor-parallel over head groups (4).
Core c handles batch b=c//4, heads 4*(c%4)..4*(c%4)+3. Each core computes
Q/K/V projections for its head slice, attention, and a partial output
projection (its heads' rows of Wo); the host sums the 4 partials per batch
and adds bo.

Device layout strategy: everything lives feature-on-partitions ("transposed")
so no on-device transposes are needed:
  - host passes X[b].T; Q^T/K^T computed as (W^T X^T) with W as stationary.
  - V computed in native [token, d] layout (X^T tiles as stationary).
  - scores computed as S^T[j, i] (key j on partitions) so the mask bias is a
    per-partition scalar and softmax normalization is deferred:
    E^T = exp(S/8 + maskbias) via one ScalarE activation (PSUM->SBUF).
  - ctx^T[d, i] = sum_j V_aug[j, d] E^T[j, i]; V_aug has a ones column so the
    softmax denominator rides along as ctx row 64.
  - normalization multiplies ctx^T by 1/denom broadcast via a tiny PE matmul.
  - out^T = Wo^T ctx^T accumulated over head pairs; host transposes back.

Schedule: 128 units (q4 in 0..3 query 512-blocks x hp head pair x jt key
tile). Per unit the two h2 score matmuls land side by side in one
[128,1024] PSUM tile (bank-aligned halves) so ONE exp activation covers the
unit; ctx matmuls for unit u-2 are emitted after unit u's scores so the
Activation engine has two units of slack. QKV/output-projection chains are
spread through the unit stream as PSUM-pool-friendly inserts.
"""

import sys

if "/opt/trn_rl_repo" not in sys.path:
    sys.path.insert(0, "/opt/trn_rl_repo")

import numpy as np

import concourse.bacc as bacc
import concourse.mybir as mybir
import concourse.tile as tile

B, N, D = 2, 2048, 1024
H, HD = 16, 64
HG = 4  # head groups (tensor parallel)
HPG = H // HG  # heads per group = 4
DG = HPG * HD  # feature slice per group = 256

F32 = mybir.dt.float32
# Matmul datapath dtype: float32r is the fast (1 cycle/row at N>=256) fp32
# matmul mode; tiles and DRAM tensors feeding matmuls must be typed fp32r.
MMT = mybir.dt.float32r
FP8 = mybir.dt.float8e4
# fp8e4 DoubleRow ctx matmuls: E and V quantize to fp8e4 and each ctx
# matmul contracts a PAIR of key tiles (K=256) in one pass. E errors
# average out over the 2048-key softmax sum; V quantization (~3.6% per
# element) is the dominant cost, landing well inside the 2e-2 gate.
FP8_CTX = True


def build_program(loop_iters: int = 1):
    nc = bacc.Bacc("TRN2", target_bir_lowering=False)

    xt = nc.dram_tensor("xt", [D, N], MMT, kind="ExternalInput")
    wq = nc.dram_tensor("wq", [128, 8, DG], MMT, kind="ExternalInput")
    wk = nc.dram_tensor("wk", [128, 8, DG], MMT, kind="ExternalInput")
    wv = nc.dram_tensor("wv", [128, 8, DG], MMT, kind="ExternalInput")
    bvr = nc.dram_tensor("bvr", [1, DG], MMT, kind="ExternalInput")
    wo = nc.dram_tensor("wo", [128, 2, D], MMT, kind="ExternalInput")
    # packed constants, one DMA: cols 0:128 ones, 128:130 bq, 130:132 bk,
    # 132:148 mask bias, 148:212 identity (rows 0-63)
    cpk = nc.dram_tensor("cpk", [128, 212], MMT, kind="ExternalInput")
    outp = nc.dram_tensor("outp", [D, N], F32, kind="ExternalOutput")

    with tile.TileContext(nc) as tc, nc.allow_low_precision(
        reason="fp32r matmul datapath; accumulation stays fp32 in PSUM"
    ):
        import contextlib

        ctx = contextlib.ExitStack()
        with ctx:
            const = ctx.enter_context(tc.tile_pool(name="const", bufs=1))
            big = ctx.enter_context(tc.tile_pool(name="big", bufs=3))
            xtcp = ctx.enter_context(tc.tile_pool(name="xtcp", bufs=4))
            qk = ctx.enter_context(tc.tile_pool(name="qk", bufs=1))
            epool = ctx.enter_context(tc.tile_pool(name="epool", bufs=4))
            rpool = ctx.enter_context(tc.tile_pool(name="rpool", bufs=2))
            # PSUM: psum_b 2 x [128,1024] (4 banks) for S tiles only;
            # psum_x 2 x [128,512] (2 banks) for chains/outproj/norm;
            # psum_c 1 x [65->128,1024] (2 banks) ctx accumulator (norm of
            # block b must drain before block b+1's first ctx matmul -- the
            # insert schedule covers that latency with chain work).
            psum_b = ctx.enter_context(
                tc.tile_pool(name="psum_b", bufs=2, space="PSUM")
            )
            psum_x = ctx.enter_context(
                tc.tile_pool(name="psum_x", bufs=2, space="PSUM")
            )
            psum_c = ctx.enter_context(
                tc.tile_pool(name="psum_c", bufs=1, space="PSUM")
            )

            loop_cm = (
                tc.For_i(0, loop_iters, 1)
                if loop_iters > 1
                else contextlib.nullcontext()
            )
            with loop_cm:
                # ---- startup DMAs. Every DMA carries ~1-2.2us of fixed
                # DGE/sem overhead, so loads are consolidated and spread
                # over the SP and Pool queues; ACT issues none (the kernel
                # end is gated by the exp drain on ACT).
                wk_sb = big.tile([128, 8, DG], MMT, tag="wk", bufs=1)
                nc.sync.dma_start(out=wk_sb[:, :, 0:128], in_=wk[:, :, 0:128])
                wq_sb = big.tile([128, 8, DG], MMT, tag="wq", bufs=1)
                nc.sync.dma_start(out=wq_sb[:, :, 0:128], in_=wq[:, :, 0:128])
                wv_sb = big.tile([128, 8, DG], MMT, tag="wv", bufs=1)
                nc.sync.dma_start(out=wv_sb, in_=wv[:, :, :])

                # X^T chunks: [128, kt=8, 512 tokens]
                xtc_t = [
                    xtcp.tile([128, 8, 512], MMT, tag="xtc", name="xtc")
                    for _ in range(4)
                ]

                def load_xtc(c, eng):
                    eng.dma_start(
                        out=xtc_t[c],
                        in_=xt[:, c * 512 : (c + 1) * 512].rearrange(
                            "(kt p) col -> p kt col", p=128
                        ),
                    )

                def load_xtc_half(c, eng, h):
                    eng.dma_start(
                        out=xtc_t[c][:, h * 4 : (h + 1) * 4, :],
                        in_=xt[
                            h * 4 * 128 : (h + 1) * 4 * 128,
                            c * 512 : (c + 1) * 512,
                        ].rearrange("(kt p) col -> p kt col", p=128),
                    )

                def load_xtc_q(c, eng, q):
                    eng.dma_start(
                        out=xtc_t[c][:, q * 2 : (q + 1) * 2, :],
                        in_=xt[
                            q * 2 * 128 : (q + 1) * 2 * 128,
                            c * 512 : (c + 1) * 512,
                        ].rearrange("(kt p) col -> p kt col", p=128),
                    )

                # chunk 0 quarters: 2 on Pool, 2 on ACT (ACT idles until
                # the first exp ~8us in; cpack rides ACT first so the bias
                # constants land by ~1us)
                cp_sb = const.tile([128, 212], MMT, tag="cpk")
                nc.scalar.dma_start(out=cp_sb, in_=cpk[:, :])
                load_xtc_q(0, nc.gpsimd, 0)
                load_xtc_q(0, nc.gpsimd, 1)
                load_xtc_q(0, nc.scalar, 2)
                load_xtc_q(0, nc.scalar, 3)
                ones = cp_sb[:, 0:128]
                bq_sb = cp_sb[:, 128:130].bitcast(F32)
                bk_sb = cp_sb[:, 130:132].bitcast(F32)
                mb_sb = cp_sb[:, 132:148].bitcast(F32)
                iden_sb = cp_sb[0:64, 148:212]
                bvr_sb = const.tile([1, DG], MMT, tag="bvr")
                nc.gpsimd.dma_start(out=bvr_sb, in_=bvr[:, :])

                if FP8_CTX:
                    # V pairs for DoubleRow: [128, jp, pair, head, 128] fp8;
                    # col 64 = ones (softmax denominator), cols 65-127 zero
                    # padding (dual-fp8 ldweights wants a pow2 column count)
                    v_sb = qk.tile([128, 8, 2, HPG, 128], FP8, tag="v")
                    nc.vector.memzero(v_sb)
                    nc.gpsimd.dma_start(
                        out=v_sb[:, :, :, :, HD : HD + 1], in_=cpk[:, 0:64]
                    )
                else:
                    # V with ones column per head: [128, jt, head, 65]
                    v_sb = qk.tile([128, 16, HPG, HD + 1], MMT, tag="v")
                    nc.gpsimd.dma_start(
                        out=v_sb[:, :, :, HD : HD + 1], in_=cpk[:, 0:64]
                    )
                wo_sb = const.tile([128, 2, D], MMT, tag="wo")

                nc.sync.dma_start(out=wk_sb[:, :, 128:DG], in_=wk[:, :, 128:DG])
                nc.sync.dma_start(out=wq_sb[:, :, 128:DG], in_=wq[:, :, 128:DG])
                load_xtc(1, nc.gpsimd)
                load_xtc(2, nc.gpsimd)
                load_xtc(3, nc.sync)
                xtc = [[xtc_t[c][:, kt, :] for kt in range(8)] for c in range(4)]

                qt_sb = [
                    qk.tile([128, N], MMT, tag=f"qt{m}", name=f"qt{m}")
                    for m in range(2)
                ]
                kt_sb = [
                    qk.tile([128, N], MMT, tag=f"kt{m}", name=f"kt{m}")
                    for m in range(2)
                ]

                # bv broadcast to all 128 partitions via PE (deferred
                # past the first chains so Pool const DMAs don't gate them)
                bv_bc = const.tile([128, DG], F32, tag="bvbc")

                def bv_bcast():
                    bv_ps = psum_x.tile([128, DG], F32, tag="x", name="bvps")
                    nc.tensor.matmul(
                        bv_ps, ones[0:1, 0:128], bvr_sb[0:1, :],
                        start=True, stop=True,
                    )
                    nc.vector.tensor_copy(bv_bc, bv_ps)

                def qk_chain(proj, hp, nt, defer_bias=False):
                    w_sb, bias_sb, dst = (
                        (wq_sb, bq_sb, qt_sb)
                        if proj == 0
                        else (wk_sb, bk_sb, kt_sb)
                    )
                    ps = psum_x.tile([128, 512], F32, tag="x", name="qkps")
                    for kt in range(8):
                        nc.tensor.matmul(
                            ps,
                            w_sb[:, kt, hp * 128 : (hp + 1) * 128],
                            xtc[nt][kt],
                            start=(kt == 0),
                            stop=(kt == 7),
                        )
                    out_ap = dst[hp][:, nt * 512 : (nt + 1) * 512]

                    def bias():
                        # deferred (boundary-cover) drains are emitted after
                        # norm_b so the DVE norm chain runs first (gpsimd
                        # cannot touch PSUM on real hw)
                        nc.vector.tensor_scalar_add(
                            out_ap, ps, bias_sb[:, hp : hp + 1]
                        )

                    if defer_bias:
                        return bias
                    nc.vector.tensor_scalar_add(
                        out_ap, ps, bias_sb[:, hp : hp + 1]
                    )

                def v_chain(mt):
                    ps = psum_x.tile([128, DG], F32, tag="x", name="vps")
                    for kt in range(8):
                        nc.tensor.matmul(
                            ps,
                            xtc[mt // 4][kt][
                                :, (mt % 4) * 128 : (mt % 4 + 1) * 128
                            ],
                            wv_sb[:, kt, :],
                            start=(kt == 0),
                            stop=(kt == 7),
                        )
                    v_out = (
                        v_sb[:, mt // 2, mt % 2, :, 0:HD]
                        if FP8_CTX
                        else v_sb[:, mt, :, 0:HD]
                    )
                    nc.vector.tensor_tensor(
                        out=v_out,
                        in0=ps.rearrange("p (h d) -> p h d", h=HPG),
                        in1=bv_bc.rearrange("p (h d) -> p h d", h=HPG),
                        op=mybir.AluOpType.add,
                    )

                # ---- unit stream ----
                ctxn = [
                    qk.tile([128, N], MMT, tag=f"ctxn{m}", name=f"ctxn{m}")
                    for m in range(2)
                ]

                blocks = [(q4, hp) for q4 in range(4) for hp in range(2)]
                units = [
                    (b_idx, q4, hp, jt)
                    for b_idx, (q4, hp) in enumerate(blocks)
                    for jt in range(16)
                ]
                ctx_ps_of = {}
                unit_e = {}

                pair_e = {}

                def emit_s_exp(u):
                    b_idx, q4, hp, jt = u
                    s_ps = psum_b.tile([128, 1024], F32, tag="bank", name="sps")
                    for h2 in range(2):
                        nc.tensor.matmul(
                            s_ps[:, h2 * 512 : (h2 + 1) * 512],
                            kt_sb[hp][
                                h2 * 64 : (h2 + 1) * 64,
                                jt * 128 : (jt + 1) * 128,
                            ],
                            qt_sb[hp][
                                h2 * 64 : (h2 + 1) * 64,
                                q4 * 512 : (q4 + 1) * 512,
                            ],
                            start=True,
                            stop=True,
                        )
                    if FP8_CTX:
                        k = jt // 2
                        if jt % 2 == 0:
                            pair_e[(b_idx, k)] = epool.tile(
                                [128, 2, 1024], FP8, tag="e", name="esb"
                            )
                        e_out = pair_e[(b_idx, k)][:, jt % 2, :]
                    else:
                        e_out = epool.tile([128, 1024], MMT, tag="e", name="esb")
                        unit_e[u] = e_out
                    nc.scalar.activation(
                        out=e_out,
                        in_=s_ps,
                        func=mybir.ActivationFunctionType.Exp,
                        bias=mb_sb[:, jt : jt + 1],
                        scale=0.125,
                    )

                def emit_ctx(u):
                    b_idx, q4, hp, jt = u
                    if b_idx not in ctx_ps_of:
                        ctx_ps_of[b_idx] = psum_c.tile(
                            [65, 1024], F32, tag="ctx", name="ctxps"
                        )
                    ctx_ps = ctx_ps_of[b_idx]
                    e_sb = unit_e.pop(u)
                    for h2 in range(2):
                        nc.tensor.matmul(
                            ctx_ps[:, h2 * 512 : (h2 + 1) * 512],
                            v_sb[:, jt, 2 * hp + h2, :],
                            e_sb[:, h2 * 512 : (h2 + 1) * 512],
                            start=(jt == 0),
                            stop=(jt == 15),
                            skip_group_check=True,
                        )

                def emit_ctx_pair(k):
                    b_idx = k // 8
                    jp = k % 8
                    q4, hp = blocks[b_idx]
                    if b_idx not in ctx_ps_of:
                        ctx_ps_of[b_idx] = psum_c.tile(
                            [128, 1024], F32, tag="ctx", name="ctxps"
                        )
                    ctx_ps = ctx_ps_of[b_idx]
                    e8 = pair_e.pop((b_idx, jp))
                    for h2 in range(2):
                        nc.tensor.matmul(
                            ctx_ps[:, h2 * 512 : (h2 + 1) * 512],
                            v_sb[:, jp, :, 2 * hp + h2, :],
                            e8[:, :, h2 * 512 : (h2 + 1) * 512],
                            start=(jp == 0),
                            stop=(jp == 7),
                            perf_mode=mybir.MatmulPerfMode.DoubleRow,
                            skip_group_check=True,
                        )

                norm_r = {}

                def emit_norm_a(b_idx):
                    # reciprocal of the denominators (row 64) -- issued to
                    # DVE right after the block's last ctx matmul so it runs
                    # while the PE chews boundary-cover work
                    ctx_ps = ctx_ps_of[b_idx]
                    r_sb = rpool.tile([65, 1024], MMT, tag="r", name="rsb")
                    for h2 in (1, 0):
                        nc.vector.reciprocal(
                            out=r_sb[64:65, h2 * 512 : (h2 + 1) * 512],
                            in_=ctx_ps[64:65, h2 * 512 : (h2 + 1) * 512],
                        )
                    norm_r[b_idx] = r_sb

                def emit_norm_b(b_idx):
                    q4, hp = blocks[b_idx]
                    ctx_ps = ctx_ps_of.pop(b_idx)
                    r_sb = norm_r.pop(b_idx)
                    # broadcast matmuls into one [64,1024] tile (a matmul
                    # output may not cross a PSUM bank: 512 f32 per half)
                    rp = psum_b.tile([64, 1024], F32, tag="bank", name="rp")
                    for h2 in (1, 0):
                        nc.tensor.matmul(
                            rp[:, h2 * 512 : (h2 + 1) * 512],
                            ones[64:65, 0:64],
                            r_sb[64:65, h2 * 512 : (h2 + 1) * 512],
                            start=True,
                            stop=True,
                            tile_position=(64, 0),
                        )
                    # rp must round-trip through SBUF (DVE may read only
                    # one PSUM input); ACT is the bottleneck engine so the
                    # copies ride DVE.
                    for h2 in (1, 0):
                        nc.vector.tensor_copy(
                            r_sb[0:64, h2 * 512 : (h2 + 1) * 512],
                            rp[:, h2 * 512 : (h2 + 1) * 512],
                        )
                    # h2=1 first so its partition-shift overlaps the h2=0
                    # multiply
                    tmp = rpool.tile([64, 512], MMT, tag="tmp", name="tmp")
                    nc.vector.tensor_tensor(
                        out=tmp,
                        in0=ctx_ps[0:64, 512:1024],
                        in1=r_sb[0:64, 512:1024],
                        op=mybir.AluOpType.mult,
                    )
                    nc.sync.dma_start(
                        out=ctxn[hp][64:128, q4 * 512 : (q4 + 1) * 512],
                        in_=tmp,
                    )
                    nc.vector.tensor_tensor(
                        out=ctxn[hp][0:64, q4 * 512 : (q4 + 1) * 512],
                        in0=ctx_ps[0:64, 0:512],
                        in1=r_sb[0:64, 0:512],
                        op=mybir.AluOpType.mult,
                    )

                def emit_outproj(q4, mo, copy_eng=None, defer_copy=False):
                    ps = psum_x.tile([128, 512], F32, tag="x", name="ops")
                    for hp in range(2):
                        nc.tensor.matmul(
                            ps,
                            wo_sb[:, hp, mo * 128 : (mo + 1) * 128],
                            ctxn[hp][:, q4 * 512 : (q4 + 1) * 512],
                            start=(hp == 0),
                            stop=(hp == 1),
                        )

                    def drain(eng=None):
                        ob = big.tile([128, 512], F32, tag="ob", name="ob")
                        (eng or copy_eng or nc.vector.tensor_copy)(ob, ps)
                        dq = nc.gpsimd if (q4 == 3 and mo % 2 == 1) else nc.sync
                        dq.dma_start(
                            out=outp[
                                mo * 128 : (mo + 1) * 128,
                                q4 * 512 : (q4 + 1) * 512,
                            ],
                            in_=ob,
                        )

                    if defer_copy:
                        return drain
                    drain()

                # ---- schedule ----
                # sched[i]: callables emitted after S(i) and ctx(i-2), in
                # order. Block boundaries (i = 16b+17): norm_a (recip on
                # DVE), cover matmuls, norm_b (broadcast + mults), then the
                # covers' deferred DVE bias/copies -- so the norm's DVE chain
                # is never queued behind cover consumers, and cover psum
                # consumers don't stall the next psum_x allocations.
                sched = {}

                def at(i, fn):
                    sched.setdefault(i, []).append(fn)

                def ch(i, proj, hp, nt):
                    at(i, lambda: qk_chain(proj, hp, nt))

                def op(i, q4, mo, eng=None):
                    at(i, lambda: emit_outproj(q4, mo, eng))

                def boundary(i, b, cover1, cover2):
                    def emit():
                        emit_norm_a(b)
                        drains = [fn() for fn in cover1]
                        emit_norm_b(b)
                        for d in drains:
                            d()
                        for fn in cover2:
                            fn()

                    at(i, emit)

                def dch(proj, hp, nt):
                    return lambda: qk_chain(proj, hp, nt, defer_bias=True)

                def dop(q4, mo):
                    return lambda: emit_outproj(q4, mo, defer_copy=True)

                at(0, bv_bcast)
                at(0, lambda: v_chain(0))
                for j in range(1, 16):
                    at(j, lambda m=j: v_chain(m))
                ch(3, 1, 0, 1)
                ch(6, 1, 0, 2)
                ch(9, 1, 0, 3)
                ch(11, 1, 1, 0)
                ch(13, 1, 1, 1)
                ch(14, 0, 1, 0)
                boundary(17, 0, [dch(1, 1, 2)], [lambda: qk_chain(1, 1, 3)])
                at(20, lambda: nc.gpsimd.dma_start(out=wo_sb, in_=wo[:, :, :]))
                ch(28, 0, 0, 1)
                boundary(33, 1, [dch(0, 1, 1)], [lambda: qk_chain(0, 0, 2)])
                op(36, 0, 0)
                op(38, 0, 1)
                boundary(
                    49, 2, [dch(0, 1, 2)],
                    [lambda: emit_outproj(0, 2), lambda: emit_outproj(0, 3)],
                )
                op(52, 0, 4)
                op(54, 0, 5)
                boundary(
                    65, 3, [dch(0, 0, 3)],
                    [lambda: emit_outproj(0, 6), lambda: emit_outproj(0, 7)],
                )
                boundary(
                    81, 4, [dch(0, 1, 3)],
                    [lambda: emit_outproj(1, 0), lambda: emit_outproj(1, 1)],
                )
                op(84, 1, 2)
                op(86, 1, 3)
                boundary(
                    97, 5, [dop(1, 4), dop(1, 5)],
                    [lambda: emit_outproj(1, 6), lambda: emit_outproj(1, 7)],
                )
                boundary(
                    113, 6, [dop(2, 0), dop(2, 1)],
                    [lambda: emit_outproj(2, 2), lambda: emit_outproj(2, 3)],
                )
                op(116, 2, 4)
                op(118, 2, 5)
                op(120, 2, 6)
                op(122, 2, 7)

                # prologue chains
                qk_chain(1, 0, 0)
                qk_chain(0, 0, 0)

                for i, u in enumerate(units):
                    emit_s_exp(u)
                    if FP8_CTX:
                        if i >= 3 and i % 2 == 1:
                            emit_ctx_pair((i - 3) // 2)
                    elif i >= 2:
                        emit_ctx(units[i - 2])
                    for fn in sched.get(i, []):
                        fn()
                if FP8_CTX:
                    emit_ctx_pair(63)
                else:
                    emit_ctx(units[-2])
                    emit_ctx(units[-1])
                emit_norm_a(7)
                emit_norm_b(7)
                for mo in range(8):
                    emit_outproj(3, mo)


    nc.finalize()
    return nc


_NC_CACHE = None


def _get_program():
    global _NC_CACHE
    if _NC_CACHE is None:
        _NC_CACHE = build_program()
    return _NC_CACHE


def make_in_maps(X, mask, Wq, bq, Wk, bk, Wv, bv, Wo, bo):
    X = np.asarray(X, dtype=np.float32)
    mask = np.asarray(mask, dtype=np.float32)
    in_maps = []
    xts = [np.ascontiguousarray(X[b].T) for b in range(B)]
    mbs = [
        np.ascontiguousarray((-1e6 * (1.0 - mask[b])).reshape(16, 128).T)
        for b in range(B)
    ]
    for c in range(8):
        b, g = c // HG, c % HG
        sl = slice(g * DG, (g + 1) * DG)
        wq_s = np.ascontiguousarray(
            np.asarray(Wq[:, sl]).reshape(8, 128, DG).transpose(1, 0, 2)
        )
        wk_s = np.ascontiguousarray(
            np.asarray(Wk[:, sl]).reshape(8, 128, DG).transpose(1, 0, 2)
        )
        wv_s = np.ascontiguousarray(
            np.asarray(Wv[:, sl]).reshape(8, 128, DG).transpose(1, 0, 2)
        )
        bq_s = np.ascontiguousarray(np.asarray(bq[sl]).reshape(2, 128).T)
        bk_s = np.ascontiguousarray(np.asarray(bk[sl]).reshape(2, 128).T)
        bv_s = np.ascontiguousarray(np.asarray(bv[sl]).reshape(1, DG))
        # Wo rows for this group, pair-packed: [64*h2+p, kt, o] = Wo[g*256+(2kt+h2)*64+p, o]
        wo_s = np.ascontiguousarray(
            np.asarray(Wo[sl, :]).reshape(2, 2, 64, D).transpose(1, 2, 0, 3)
            .reshape(128, 2, D)
        )
        cpk = np.zeros((128, 212), dtype=np.float32)
        cpk[:, 0:128] = 1.0
        cpk[:, 128:130] = bq_s
        cpk[:, 130:132] = bk_s
        cpk[:, 132:148] = mbs[b]
        cpk[0:64, 148:212] = np.eye(64, dtype=np.float32)
        in_maps.append(
            {
                "xt": xts[b],
                "cpk": cpk,
                "wq": wq_s.astype(np.float32),
                "wk": wk_s.astype(np.float32),
                "wv": wv_s.astype(np.float32),
                "bvr": bv_s.astype(np.float32),
                "wo": wo_s.astype(np.float32),
            }
        )
    return in_maps


def gather_output(results, bo):
    out = np.zeros((B, N, D), dtype=np.float32)
    for c in range(8):
        out[c // HG] += results[c]["outp"].T
    out += np.asarray(bo, dtype=np.float32)
    return out


def kernel(**inputs):
    from concourse import bass_utils

    nc = _get_program()
    in_maps = make_in_maps(**inputs)
    res = bass_utils.run_bass_kernel_spmd(nc, in_maps, core_ids=list(range(8)))
    return gather_output(res.results, inputs["bo"])


# revision 30
# speedup vs baseline: 1.0013x; 1.0013x over previous
"""Multi-head attention (B=2, N=2048, D=1024, H=16, HD=64) on 8 trn2 NeuronCores.

Sharding: data-parallel over batch (2) x tensor-parallel over head groups (4).
Core c handles batch b=c//4, heads 4*(c%4)..4*(c%4)+3. Each core computes
Q/K/V projections for its head slice, attention, and a partial output
projection (its heads' rows of Wo); the host sums the 4 partials per batch
and adds bo.

Device layout strategy: everything lives feature-on-partitions ("transposed")
so no on-device transposes are needed:
  - host passes X[b].T; Q^T/K^T computed as (W^T X^T) with W as stationary.
  - V computed in native [token, d] layout (X^T tiles as stationary).
  - scores computed as S^T[j, i] (key j on partitions) so the mask bias is a
    per-partition scalar and softmax normalization is deferred:
    E^T = exp(S/8 + maskbias) via one ScalarE activation (PSUM->SBUF).
  - ctx^T[d, i] = sum_j V_aug[j, d] E^T[j, i]; V_aug has a ones column so the
    softmax denominator rides along as ctx row 64.
  - normalization multiplies ctx^T by 1/denom broadcast via a tiny PE matmul.
  - out^T = Wo^T ctx^T accumulated over head pairs; host transposes back.

Schedule: 128 units (q4 in 0..3 query 512-blocks x hp head pair x jt key
tile). Per unit the two h2 score matmuls land side by side in one
[128,1024] PSUM tile (bank-aligned halves) so ONE exp activation covers the
unit; ctx matmuls for unit u-2 are emitted after unit u's scores so the
Activation engine has two units of slack. QKV/output-projection chains are
spread through the unit stream as PSUM-pool-friendly inserts.
"""

import sys

if "/opt/trn_rl_repo" not in sys.path:
    sys.path.insert(0, "/opt/trn_rl_repo")

import numpy as np

import concourse.bacc as bacc
import concourse.mybir as mybir
import concourse.tile as tile

B, N, D = 2, 2048, 1024
H, HD = 16, 64
HG = 4  # head groups (tensor parallel)
HPG = H // HG  # heads per group = 4
DG = HPG * HD  # feature slice per group = 256

F32 = mybir.dt.float32
# Matmul datapath dtype: float32r is the fast (1 cycle/row at N>=256) fp32
# matmul mode; tiles and DRAM tensors feeding matmuls must be typed fp32r.
MMT = mybir.dt.float32r
FP8 = mybir.dt.float8e4
# fp8e4 DoubleRow ctx matmuls: E and V quantize to fp8e4 and each ctx
# matmul contracts a PAIR of key tiles (K=256) in one pass. E errors
# average out over the 2048-key softmax sum; V quantization (~3.6% per
# element) is the dominant cost, landing well inside the 2e-2 gate.
FP8_CTX = True


def build_program(loop_iters: int = 1):
    nc = bacc.Bacc("TRN2", target_bir_lowering=False)

    xt = nc.dram_tensor("xt", [D, N], MMT, kind="ExternalInput")
    wq = nc.dram_tensor("wq", [128, 8, DG], MMT, kind="ExternalInput")
    wk = nc.dram_tensor("wk", [128, 8, DG], MMT, kind="ExternalInput")
    wv = nc.dram_tensor("wv", [128, 8, DG], MMT, kind="ExternalInput")
    bvr = nc.dram_tensor("bvr", [1, DG], MMT, kind="ExternalInput")
    wo = nc.dram_tensor("wo", [128, 2, D], MMT, kind="ExternalInput")
    # packed constants, one DMA: cols 0:128 ones, 128:130 bq, 130:132 bk,
    # 132:148 mask bias, 148:212 identity (rows 0-63)
    cpk = nc.dram_tensor("cpk", [128, 212], MMT, kind="ExternalInput")
    outp = nc.dram_tensor("outp", [D, N], F32, kind="ExternalOutput")

    with tile.TileContext(nc) as tc, nc.allow_low_precision(
        reason="fp32r matmul datapath; accumulation stays fp32 in PSUM"
    ):
        import contextlib

        ctx = contextlib.ExitStack()
        with ctx:
            const = ctx.enter_context(tc.tile_pool(name="const", bufs=1))
            big = ctx.enter_context(tc.tile_pool(name="big", bufs=3))
            xtcp = ctx.enter_context(tc.tile_pool(name="xtcp", bufs=4))
            qk = ctx.enter_context(tc.tile_pool(name="qk", bufs=1))
            epool = ctx.enter_context(tc.tile_pool(name="epool", bufs=4))
            rpool = ctx.enter_context(tc.tile_pool(name="rpool", bufs=2))
            # PSUM: psum_b 3 x [128,1024] (6 banks) shared by S tiles,
            # chains, outproj and norm broadcasts -- the 3-deep rotation
            # decouples the S->exp pipeline from slow chain/norm drains;
            # psum_c 1 x [128,1024] (2 banks) ctx accumulator (norm of
            # block b must drain before block b+1's first ctx matmul -- the
            # insert schedule covers that latency with chain work).
            psum_b = ctx.enter_context(
                tc.tile_pool(name="psum_b", bufs=2, space="PSUM")
            )
            psum_x = ctx.enter_context(
                tc.tile_pool(name="psum_x", bufs=2, space="PSUM")
            )
            psum_c = ctx.enter_context(
                tc.tile_pool(name="psum_c", bufs=1, space="PSUM")
            )

            loop_cm = (
                tc.For_i(0, loop_iters, 1)
                if loop_iters > 1
                else contextlib.nullcontext()
            )
            with loop_cm:
                # ---- startup DMAs. Every DMA carries ~1-2.2us of fixed
                # DGE/sem overhead, so loads are consolidated and spread
                # over the SP and Pool queues; ACT issues none (the kernel
                # end is gated by the exp drain on ACT).
                wk_sb = big.tile([128, 8, DG], MMT, tag="wk", bufs=1)
                nc.sync.dma_start(out=wk_sb[:, :, 0:128], in_=wk[:, :, 0:128])
                wq_sb = big.tile([128, 8, DG], MMT, tag="wq", bufs=1)
                nc.sync.dma_start(out=wq_sb[:, :, 0:128], in_=wq[:, :, 0:128])
                wv_sb = big.tile([128, 8, DG], MMT, tag="wv", bufs=1)
                nc.sync.dma_start(out=wv_sb, in_=wv[:, :, :])

                # X^T chunks: [128, kt=8, 512 tokens]
                xtc_t = [
                    xtcp.tile([128, 8, 512], MMT, tag="xtc", name="xtc")
                    for _ in range(4)
                ]

                def load_xtc(c, eng):
                    eng.dma_start(
                        out=xtc_t[c],
                        in_=xt[:, c * 512 : (c + 1) * 512].rearrange(
                            "(kt p) col -> p kt col", p=128
                        ),
                    )

                def load_xtc_half(c, eng, h):
                    eng.dma_start(
                        out=xtc_t[c][:, h * 4 : (h + 1) * 4, :],
                        in_=xt[
                            h * 4 * 128 : (h + 1) * 4 * 128,
                            c * 512 : (c + 1) * 512,
                        ].rearrange("(kt p) col -> p kt col", p=128),
                    )

                def load_xtc_q(c, eng, q):
                    eng.dma_start(
                        out=xtc_t[c][:, q * 2 : (q + 1) * 2, :],
                        in_=xt[
                            q * 2 * 128 : (q + 1) * 2 * 128,
                            c * 512 : (c + 1) * 512,
                        ].rearrange("(kt p) col -> p kt col", p=128),
                    )

                # chunk 0 quarters: 2 on Pool, 2 on ACT (ACT idles until
                # the first exp ~8us in; cpack rides ACT first so the bias
                # constants land by ~1us)
                cp_sb = const.tile([128, 212], MMT, tag="cpk")
                nc.scalar.dma_start(out=cp_sb, in_=cpk[:, :])
                load_xtc_q(0, nc.gpsimd, 0)
                load_xtc_q(0, nc.gpsimd, 1)
                load_xtc_q(0, nc.scalar, 2)
                load_xtc_q(0, nc.scalar, 3)
                ones = cp_sb[:, 0:128]
                bq_sb = cp_sb[:, 128:130].bitcast(F32)
                bk_sb = cp_sb[:, 130:132].bitcast(F32)
                mb_sb = cp_sb[:, 132:148].bitcast(F32)
                iden_sb = cp_sb[0:64, 148:212]
                bvr_sb = const.tile([1, DG], MMT, tag="bvr")
                nc.gpsimd.dma_start(out=bvr_sb, in_=bvr[:, :])

                if FP8_CTX:
                    # V pairs for DoubleRow: [128, jp, pair, head, 128] fp8;
                    # col 64 = ones (softmax denominator), cols 65-127 zero
                    # padding (dual-fp8 ldweights wants a pow2 column count)
                    v_sb = qk.tile([128, 8, 2, HPG, 128], FP8, tag="v")
                    nc.vector.memzero(v_sb)
                    nc.gpsimd.dma_start(
                        out=v_sb[:, :, :, :, HD : HD + 1], in_=cpk[:, 0:64]
                    )
                else:
                    # V with ones column per head: [128, jt, head, 65]
                    v_sb = qk.tile([128, 16, HPG, HD + 1], MMT, tag="v")
                    nc.gpsimd.dma_start(
                        out=v_sb[:, :, :, HD : HD + 1], in_=cpk[:, 0:64]
                    )
                wo_sb = const.tile([128, 2, D], MMT, tag="wo")

                nc.sync.dma_start(out=wk_sb[:, :, 128:DG], in_=wk[:, :, 128:DG])
                nc.sync.dma_start(out=wq_sb[:, :, 128:DG], in_=wq[:, :, 128:DG])
                load_xtc(1, nc.gpsimd)
                load_xtc(2, nc.gpsimd)
                load_xtc(3, nc.sync)
                xtc = [[xtc_t[c][:, kt, :] for kt in range(8)] for c in range(4)]

                qt_sb = [
                    qk.tile([128, N], MMT, tag=f"qt{m}", name=f"qt{m}")
                    for m in range(2)
                ]
                kt_sb = [
                    qk.tile([128, N], MMT, tag=f"kt{m}", name=f"kt{m}")
                    for m in range(2)
                ]

                # bv broadcast to all 128 partitions via PE (deferred
                # past the first chains so Pool const DMAs don't gate them)
                bv_bc = const.tile([128, DG], F32, tag="bvbc")

                def bv_bcast():
                    bv_ps = psum_x.tile([128, DG], F32, tag="x", name="bvps")
                    nc.tensor.matmul(
                        bv_ps, ones[0:1, 0:128], bvr_sb[0:1, :],
                        start=True, stop=True,
                    )
                    nc.vector.tensor_copy(bv_bc, bv_ps)

                def qk_chain(proj, hp, nt, defer_bias=False):
                    w_sb, bias_sb, dst = (
                        (wq_sb, bq_sb, qt_sb)
                        if proj == 0
                        else (wk_sb, bk_sb, kt_sb)
                    )
                    ps = psum_x.tile([128, 512], F32, tag="x", name="qkps")
                    for kt in range(8):
                        nc.tensor.matmul(
                            ps,
                            w_sb[:, kt, hp * 128 : (hp + 1) * 128],
                            xtc[nt][kt],
                            start=(kt == 0),
                            stop=(kt == 7),
                        )
                    out_ap = dst[hp][:, nt * 512 : (nt + 1) * 512]

                    def bias():
                        # deferred (boundary-cover) drains are emitted after
                        # norm_b so the DVE norm chain runs first (gpsimd
                        # cannot touch PSUM on real hw)
                        nc.vector.tensor_scalar_add(
                            out_ap, ps, bias_sb[:, hp : hp + 1]
                        )

                    if defer_bias:
                        return bias
                    nc.vector.tensor_scalar_add(
                        out_ap, ps, bias_sb[:, hp : hp + 1]
                    )

                def v_chain(mt):
                    ps = psum_x.tile([128, DG], F32, tag="x", name="vps")
                    for kt in range(8):
                        nc.tensor.matmul(
                            ps,
                            xtc[mt // 4][kt][
                                :, (mt % 4) * 128 : (mt % 4 + 1) * 128
                            ],
                            wv_sb[:, kt, :],
                            start=(kt == 0),
                            stop=(kt == 7),
                        )
                    v_out = (
                        v_sb[:, mt // 2, mt % 2, :, 0:HD]
                        if FP8_CTX
                        else v_sb[:, mt, :, 0:HD]
                    )
                    nc.vector.tensor_tensor(
                        out=v_out,
                        in0=ps.rearrange("p (h d) -> p h d", h=HPG),
                        in1=bv_bc.rearrange("p (h d) -> p h d", h=HPG),
                        op=mybir.AluOpType.add,
                    )

                # ---- unit stream ----
                ctxn = [
                    qk.tile([128, N], MMT, tag=f"ctxn{m}", name=f"ctxn{m}")
                    for m in range(2)
                ]

                blocks = [(q4, hp) for q4 in range(4) for hp in range(2)]
                units = [
                    (b_idx, q4, hp, jt)
                    for b_idx, (q4, hp) in enumerate(blocks)
                    for jt in range(16)
                ]
                ctx_ps_of = {}
                unit_e = {}

                pair_e = {}

                def emit_s_exp(u):
                    b_idx, q4, hp, jt = u
                    s_ps = psum_b.tile([128, 1024], F32, tag="bank", name="sps")
                    for h2 in range(2):
                        nc.tensor.matmul(
                            s_ps[:, h2 * 512 : (h2 + 1) * 512],
                            kt_sb[hp][
                                h2 * 64 : (h2 + 1) * 64,
                                jt * 128 : (jt + 1) * 128,
                            ],
                            qt_sb[hp][
                                h2 * 64 : (h2 + 1) * 64,
                                q4 * 512 : (q4 + 1) * 512,
                            ],
                            start=True,
                            stop=True,
                        )
                    if FP8_CTX:
                        k = jt // 2
                        if jt % 2 == 0:
                            pair_e[(b_idx, k)] = epool.tile(
                                [128, 2, 1024], FP8, tag="e", name="esb"
                            )
                        e_out = pair_e[(b_idx, k)][:, jt % 2, :]
                    else:
                        e_out = epool.tile([128, 1024], MMT, tag="e", name="esb")
                        unit_e[u] = e_out
                    nc.scalar.activation(
                        out=e_out,
                        in_=s_ps,
                        func=mybir.ActivationFunctionType.Exp,
                        bias=mb_sb[:, jt : jt + 1],
                        scale=0.125,
                    )

                def emit_ctx(u):
                    b_idx, q4, hp, jt = u
                    if b_idx not in ctx_ps_of:
                        ctx_ps_of[b_idx] = psum_c.tile(
                            [65, 1024], F32, tag="ctx", name="ctxps"
                        )
                    ctx_ps = ctx_ps_of[b_idx]
                    e_sb = unit_e.pop(u)
                    for h2 in range(2):
                        nc.tensor.matmul(
                            ctx_ps[:, h2 * 512 : (h2 + 1) * 512],
                            v_sb[:, jt, 2 * hp + h2, :],
                            e_sb[:, h2 * 512 : (h2 + 1) * 512],
                            start=(jt == 0),
                            stop=(jt == 15),
                            skip_group_check=True,
                        )

                def emit_ctx_pair(k):
                    b_idx = k // 8
                    jp = k % 8
                    q4, hp = blocks[b_idx]
                    if b_idx not in ctx_ps_of:
                        ctx_ps_of[b_idx] = psum_c.tile(
                            [128, 1024], F32, tag="ctx", name="ctxps"
                        )
                    ctx_ps = ctx_ps_of[b_idx]
                    e8 = pair_e.pop((b_idx, jp))
                    for h2 in range(2):
                        nc.tensor.matmul(
                            ctx_ps[:, h2 * 512 : (h2 + 1) * 512],
                            v_sb[:, jp, :, 2 * hp + h2, :],
                            e8[:, :, h2 * 512 : (h2 + 1) * 512],
                            start=(jp == 0),
                            stop=(jp == 7),
                            perf_mode=mybir.MatmulPerfMode.DoubleRow,
                            skip_group_check=True,
                        )

                norm_r = {}

                def emit_norm_a(b_idx):
                    # reciprocal of the denominators (row 64) -- issued to
                    # DVE right after the block's last ctx matmul so it runs
                    # while the PE chews boundary-cover work
                    ctx_ps = ctx_ps_of[b_idx]
                    r_sb = rpool.tile([65, 1024], MMT, tag="r", name="rsb")
                    for h2 in (1, 0):
                        nc.vector.reciprocal(
                            out=r_sb[64:65, h2 * 512 : (h2 + 1) * 512],
                            in_=ctx_ps[64:65, h2 * 512 : (h2 + 1) * 512],
                        )
                    norm_r[b_idx] = r_sb

                def emit_norm_b(b_idx, copy_eng=None):
                    q4, hp = blocks[b_idx]
                    ctx_ps = ctx_ps_of.pop(b_idx)
                    r_sb = norm_r.pop(b_idx)
                    # broadcast matmuls into one [64,1024] tile (a matmul
                    # output may not cross a PSUM bank: 512 f32 per half)
                    rp = psum_b.tile([64, 1024], F32, tag="bank", name="rp")
                    for h2 in (1, 0):
                        nc.tensor.matmul(
                            rp[:, h2 * 512 : (h2 + 1) * 512],
                            ones[64:65, 0:64],
                            r_sb[64:65, h2 * 512 : (h2 + 1) * 512],
                            start=True,
                            stop=True,
                            tile_position=(64, 0),
                        )
                    # rp must round-trip through SBUF (DVE may read only
                    # one PSUM input); ACT is the bottleneck engine so the
                    # copies ride DVE.
                    for h2 in (1, 0):
                        (copy_eng or nc.vector.tensor_copy)(
                            r_sb[0:64, h2 * 512 : (h2 + 1) * 512],
                            rp[:, h2 * 512 : (h2 + 1) * 512],
                        )
                    # h2=1 first so its partition-shift overlaps the h2=0
                    # multiply
                    tmp = rpool.tile([64, 512], MMT, tag="tmp", name="tmp")
                    nc.vector.tensor_tensor(
                        out=tmp,
                        in0=ctx_ps[0:64, 512:1024],
                        in1=r_sb[0:64, 512:1024],
                        op=mybir.AluOpType.mult,
                    )
                    nc.sync.dma_start(
                        out=ctxn[hp][64:128, q4 * 512 : (q4 + 1) * 512],
                        in_=tmp,
                    )
                    nc.vector.tensor_tensor(
                        out=ctxn[hp][0:64, q4 * 512 : (q4 + 1) * 512],
                        in0=ctx_ps[0:64, 0:512],
                        in1=r_sb[0:64, 0:512],
                        op=mybir.AluOpType.mult,
                    )

                def emit_outproj(q4, mo, copy_eng=None, defer_copy=False):
                    ps = psum_x.tile([128, 512], F32, tag="x", name="ops")
                    for hp in range(2):
                        nc.tensor.matmul(
                            ps,
                            wo_sb[:, hp, mo * 128 : (mo + 1) * 128],
                            ctxn[hp][:, q4 * 512 : (q4 + 1) * 512],
                            start=(hp == 0),
                            stop=(hp == 1),
                        )

                    def drain(eng=None):
                        ob = big.tile([128, 512], F32, tag="ob", name="ob")
                        (eng or copy_eng or nc.vector.tensor_copy)(ob, ps)
                        dq = nc.gpsimd if (q4 == 3 and mo % 2 == 1) else nc.sync
                        dq.dma_start(
                            out=outp[
                                mo * 128 : (mo + 1) * 128,
                                q4 * 512 : (q4 + 1) * 512,
                            ],
                            in_=ob,
                        )

                    if defer_copy:
                        return drain
                    drain()

                # ---- schedule ----
                # sched[i]: callables emitted after S(i) and ctx(i-2), in
                # order. Block boundaries (i = 16b+17): norm_a (recip on
                # DVE), cover matmuls, norm_b (broadcast + mults), then the
                # covers' deferred DVE bias/copies -- so the norm's DVE chain
                # is never queued behind cover consumers, and cover psum
                # consumers don't stall the next psum_x allocations.
                sched = {}

                def at(i, fn):
                    sched.setdefault(i, []).append(fn)

                def ch(i, proj, hp, nt):
                    at(i, lambda: qk_chain(proj, hp, nt))

                def op(i, q4, mo, eng=None):
                    at(i, lambda: emit_outproj(q4, mo, eng))

                def boundary(i, b, cover1, cover2):
                    def emit():
                        emit_norm_a(b)
                        drains = [fn() for fn in cover1]
                        emit_norm_b(b)
                        for d in drains:
                            d()

                    at(i, emit)
                    # cover2 lands one unit later so S(i+1) (and its exp)
                    # isn't queued behind the whole boundary burst
                    for fn in cover2:
                        at(i + 1, fn)

                def dch(proj, hp, nt):
                    return lambda: qk_chain(proj, hp, nt, defer_bias=True)

                def dop(q4, mo):
                    return lambda: emit_outproj(q4, mo, defer_copy=True)

                at(0, bv_bcast)
                at(0, lambda: v_chain(0))
                for j in range(1, 16):
                    at(j, lambda m=j: v_chain(m))
                ch(3, 1, 0, 1)
                ch(6, 1, 0, 2)
                ch(9, 1, 0, 3)
                ch(11, 1, 1, 0)
                ch(13, 1, 1, 1)
                ch(14, 0, 1, 0)
                boundary(17, 0, [dch(1, 1, 2)], [lambda: qk_chain(1, 1, 3)])
                at(20, lambda: nc.gpsimd.dma_start(out=wo_sb, in_=wo[:, :, :]))
                ch(28, 0, 0, 1)
                boundary(33, 1, [dch(0, 1, 1)], [lambda: qk_chain(0, 0, 2)])
                op(36, 0, 0)
                op(38, 0, 1)
                boundary(
                    49, 2, [dch(0, 1, 2)],
                    [lambda: emit_outproj(0, 2), lambda: emit_outproj(0, 3)],
                )
                op(52, 0, 4)
                op(54, 0, 5)
                boundary(
                    65, 3, [dch(0, 0, 3)],
                    [lambda: emit_outproj(0, 6), lambda: emit_outproj(0, 7)],
                )
                boundary(
                    81, 4, [dch(0, 1, 3)],
                    [lambda: emit_outproj(1, 0), lambda: emit_outproj(1, 1)],
                )
                op(84, 1, 2)
                op(86, 1, 3)
                boundary(
                    97, 5, [dop(1, 4), dop(1, 5)],
                    [lambda: emit_outproj(1, 6), lambda: emit_outproj(1, 7)],
                )
                boundary(
                    113, 6, [dop(2, 0), dop(2, 1)],
                    [lambda: emit_outproj(2, 2), lambda: emit_outproj(2, 3)],
                )
                op(116, 2, 4)
                op(118, 2, 5)
                op(120, 2, 6)
                op(122, 2, 7)

                # prologue chains
                qk_chain(1, 0, 0)
                qk_chain(0, 0, 0)

                for i, u in enumerate(units):
                    emit_s_exp(u)
                    if FP8_CTX:
                        if i >= 3 and i % 2 == 1:
                            emit_ctx_pair((i - 3) // 2)
                    elif i >= 2:
                        emit_ctx(units[i - 2])
                    for fn in sched.get(i, []):
                        fn()
                if FP8_CTX:
                    emit_ctx_pair(63)
                else:
                    emit_ctx(units[-2])
                    emit_ctx(units[-1])
                emit_norm_a(7)
                emit_norm_b(7, copy_eng=nc.scalar.copy)
                for mo in range(8):
                    emit_outproj(
                        3, mo, nc.scalar.copy if mo % 2 == 0 else None
                    )


    nc.finalize()
    return nc


_NC_CACHE = None


def _get_program():
    global _NC_CACHE
    if _NC_CACHE is None:
        _NC_CACHE = build_program()
    return _NC_CACHE


def make_in_maps(X, mask, Wq, bq, Wk, bk, Wv, bv, Wo, bo):
    X = np.asarray(X, dtype=np.float32)
    mask = np.asarray(mask, dtype=np.float32)
    in_maps = []
    xts = [np.ascontiguousarray(X[b].T) for b in range(B)]
    mbs = [
        np.ascontiguousarray((-1e6 * (1.0 - mask[b])).reshape(16, 128).T)
        for b in range(B)
    ]
    for c in range(8):
        b, g = c // HG, c % HG
        sl = slice(g * DG, (g + 1) * DG)
        wq_s = np.ascontiguousarray(
            np.asarray(Wq[:, sl]).reshape(8, 128, DG).transpose(1, 0, 2)
        )
        wk_s = np.ascontiguousarray(
            np.asarray(Wk[:, sl]).reshape(8, 128, DG).transpose(1, 0, 2)
        )
        wv_s = np.ascontiguousarray(
            np.asarray(Wv[:, sl]).reshape(8, 128, DG).transpose(1, 0, 2)
        )
        bq_s = np.ascontiguousarray(np.asarray(bq[sl]).reshape(2, 128).T)
        bk_s = np.ascontiguousarray(np.asarray(bk[sl]).reshape(2, 128).T)
        bv_s = np.ascontiguousarray(np.asarray(bv[sl]).reshape(1, DG))
        # Wo rows for this group, pair-packed: [64*h2+p, kt, o] = Wo[g*256+(2kt+h2)*64+p, o]
        wo_s = np.ascontiguousarray(
            np.asarray(Wo[sl, :]).reshape(2, 2, 64, D).transpose(1, 2, 0, 3)
            .reshape(128, 2, D)
        )
        cpk = np.zeros((128, 212), dtype=np.float32)
        cpk[:, 0:128] = 1.0
        cpk[:, 128:130] = bq_s
        cpk[:, 130:132] = bk_s
        cpk[:, 132:148] = mbs[b]
        cpk[0:64, 148:212] = np.eye(64, dtype=np.float32)
        in_maps.append(
            {
                "xt": xts[b],
                "cpk": cpk,
                "wq": wq_s.astype(np.float32),
                "wk": wk_s.astype(np.float32),
                "wv": wv_s.astype(np.float32),
                "bvr": bv_s.astype(np.float32),
                "wo": wo_s.astype(np.float32),
            }
        )
    return in_maps


def gather_output(results, bo):
    out = np.zeros((B, N, D), dtype=np.float32)
    for c in range(8):
        out[c // HG] += results[c]["outp"].T
    out += np.asarray(bo, dtype=np.float32)
    return out


def kernel(**inputs):
    from concourse import bass_utils

    nc = _get_program()
    in_maps = make_in_maps(**inputs)
    res = bass_utils.run_bass_kernel_spmd(nc, in_maps, core_ids=list(range(8)))
    return gather_output(res.results, inputs["bo"])


# revision 34
# speedup vs baseline: 1.3443x; 1.3426x over previous
"""Multi-head attention (B=2, N=2048, D=1024, H=16, HD=64) on 8 trn2 NeuronCores.

Sharding: data-parallel over batch (2) x tensor-parallel over head groups (4).
Core c handles batch b=c//4, heads 4*(c%4)..4*(c%4)+3. Each core computes
Q/K/V projections for its head slice, attention, and a partial output
projection (its heads' rows of Wo); the host sums the 4 partials per batch
and adds bo.

Device layout strategy: everything lives feature-on-partitions ("transposed")
so no on-device transposes are needed:
  - host passes X[b].T; Q^T/K^T computed as (W^T X^T) with W as stationary.
  - V computed in native [token, d] layout (X^T tiles as stationary).
  - scores computed as S^T[j, i] (key j on partitions) so the mask bias is a
    per-partition scalar and softmax normalization is deferred:
    E^T = exp(S/8 + maskbias) via one ScalarE activation (PSUM->SBUF).
  - ctx^T[d, i] = sum_j V_aug[j, d] E^T[j, i]; V_aug has a ones column so the
    softmax denominator rides along as ctx row 64.
  - normalization multiplies ctx^T by 1/denom broadcast via a tiny PE matmul.
  - out^T = Wo^T ctx^T accumulated over head pairs; host transposes back.

Schedule: 128 units (q4 in 0..3 query 512-blocks x hp head pair x jt key
tile). Per unit the two h2 score matmuls land side by side in one
[128,1024] PSUM tile (bank-aligned halves) so ONE exp activation covers the
unit; ctx matmuls for unit u-2 are emitted after unit u's scores so the
Activation engine has two units of slack. QKV/output-projection chains are
spread through the unit stream as PSUM-pool-friendly inserts.
"""

import sys

if "/opt/trn_rl_repo" not in sys.path:
    sys.path.insert(0, "/opt/trn_rl_repo")

import numpy as np

import concourse.bacc as bacc
import concourse.mybir as mybir
import concourse.tile as tile

B, N, D = 2, 2048, 1024
H, HD = 16, 64
HG = 4  # head groups (tensor parallel)
HPG = H // HG  # heads per group = 4
DG = HPG * HD  # feature slice per group = 256

F32 = mybir.dt.float32
# Matmul datapath dtype: float32r is the fast (1 cycle/row at N>=256) fp32
# matmul mode; tiles and DRAM tensors feeding matmuls must be typed fp32r.
MMT = mybir.dt.float32r
FP8 = mybir.dt.float8e4
# fp8e4 DoubleRow ctx matmuls: E and V quantize to fp8e4 and each ctx
# matmul contracts a PAIR of key tiles (K=256) in one pass. E errors
# average out over the 2048-key softmax sum; V quantization (~3.6% per
# element) is the dominant cost, landing well inside the 2e-2 gate.
FP8_CTX = True


def build_program(loop_iters: int = 1):
    nc = bacc.Bacc("TRN2", target_bir_lowering=False)

    xt = nc.dram_tensor("xt", [D, N], MMT, kind="ExternalInput")
    wq = nc.dram_tensor("wq", [128, 8, DG], MMT, kind="ExternalInput")
    wk = nc.dram_tensor("wk", [128, 8, DG], MMT, kind="ExternalInput")
    wv = nc.dram_tensor("wv", [128, 8, DG], MMT, kind="ExternalInput")
    bvr = nc.dram_tensor("bvr", [1, DG], MMT, kind="ExternalInput")
    wo = nc.dram_tensor("wo", [128, 2, D], MMT, kind="ExternalInput")
    # packed constants, one DMA: cols 0:128 ones, 128:130 bq, 130:132 bk,
    # 132:148 mask bias, 148:212 identity (rows 0-63)
    cpk = nc.dram_tensor("cpk", [128, 212], MMT, kind="ExternalInput")
    outp = nc.dram_tensor("outp", [D, N], F32, kind="ExternalOutput")

    with tile.TileContext(nc) as tc, nc.allow_low_precision(
        reason="fp32r matmul datapath; accumulation stays fp32 in PSUM"
    ):
        import contextlib

        ctx = contextlib.ExitStack()
        with ctx:
            const = ctx.enter_context(tc.tile_pool(name="const", bufs=1))
            big = ctx.enter_context(tc.tile_pool(name="big", bufs=3))
            xtcp = ctx.enter_context(tc.tile_pool(name="xtcp", bufs=4))
            qk = ctx.enter_context(tc.tile_pool(name="qk", bufs=1))
            epool = ctx.enter_context(tc.tile_pool(name="epool", bufs=4))
            rpool = ctx.enter_context(tc.tile_pool(name="rpool", bufs=2))
            # PSUM: psum_b 3 x [128,1024] (6 banks) shared by S tiles,
            # chains, outproj and norm broadcasts -- the 3-deep rotation
            # decouples the S->exp pipeline from slow chain/norm drains;
            # psum_c 1 x [128,1024] (2 banks) ctx accumulator (norm of
            # block b must drain before block b+1's first ctx matmul -- the
            # insert schedule covers that latency with chain work).
            psum_b = ctx.enter_context(
                tc.tile_pool(name="psum_b", bufs=2, space="PSUM")
            )
            psum_x = ctx.enter_context(
                tc.tile_pool(name="psum_x", bufs=2, space="PSUM")
            )
            psum_c = ctx.enter_context(
                tc.tile_pool(name="psum_c", bufs=1, space="PSUM")
            )

            loop_cm = (
                tc.For_i(0, loop_iters, 1)
                if loop_iters > 1
                else contextlib.nullcontext()
            )
            with loop_cm:
                # ---- startup DMAs. Every DMA carries ~1-2.2us of fixed
                # DGE/sem overhead, so loads are consolidated and spread
                # over the SP and Pool queues; ACT issues none (the kernel
                # end is gated by the exp drain on ACT).
                wk_sb = big.tile([128, 8, DG], MMT, tag="wk", bufs=1)
                nc.sync.dma_start(out=wk_sb[:, :, 0:128], in_=wk[:, :, 0:128])
                wq_sb = big.tile([128, 8, DG], MMT, tag="wq", bufs=1)
                nc.sync.dma_start(out=wq_sb[:, :, 0:128], in_=wq[:, :, 0:128])
                wv_sb = big.tile([128, 8, DG], MMT, tag="wv", bufs=1)
                nc.sync.dma_start(out=wv_sb, in_=wv[:, :, :])

                # X^T chunks: [128, kt=8, 512 tokens]
                xtc_t = [
                    xtcp.tile([128, 8, 512], MMT, tag="xtc", name="xtc")
                    for _ in range(4)
                ]

                def load_xtc(c, eng):
                    eng.dma_start(
                        out=xtc_t[c],
                        in_=xt[:, c * 512 : (c + 1) * 512].rearrange(
                            "(kt p) col -> p kt col", p=128
                        ),
                    )

                def load_xtc_half(c, eng, h):
                    eng.dma_start(
                        out=xtc_t[c][:, h * 4 : (h + 1) * 4, :],
                        in_=xt[
                            h * 4 * 128 : (h + 1) * 4 * 128,
                            c * 512 : (c + 1) * 512,
                        ].rearrange("(kt p) col -> p kt col", p=128),
                    )

                def load_xtc_q(c, eng, q):
                    eng.dma_start(
                        out=xtc_t[c][:, q * 2 : (q + 1) * 2, :],
                        in_=xt[
                            q * 2 * 128 : (q + 1) * 2 * 128,
                            c * 512 : (c + 1) * 512,
                        ].rearrange("(kt p) col -> p kt col", p=128),
                    )

                # chunk 0 quarters: 2 on Pool, 2 on ACT (ACT idles until
                # the first exp ~8us in; cpack rides ACT first so the bias
                # constants land by ~1us)
                cp_sb = const.tile([128, 212], MMT, tag="cpk")
                nc.scalar.dma_start(out=cp_sb, in_=cpk[:, :])
                load_xtc_q(0, nc.gpsimd, 0)
                load_xtc_q(0, nc.gpsimd, 1)
                load_xtc_q(0, nc.scalar, 2)
                load_xtc_q(0, nc.scalar, 3)
                ones = cp_sb[:, 0:128]
                bq_sb = cp_sb[:, 128:130].bitcast(F32)
                bk_sb = cp_sb[:, 130:132].bitcast(F32)
                mb_sb = cp_sb[:, 132:148].bitcast(F32)
                iden_sb = cp_sb[0:64, 148:212]
                bvr_sb = const.tile([1, DG], MMT, tag="bvr")
                nc.gpsimd.dma_start(out=bvr_sb, in_=bvr[:, :])

                if FP8_CTX:
                    # V pairs for DoubleRow: [128, jp, pair, head, 128] fp8;
                    # col 64 = ones (softmax denominator), cols 65-127 zero
                    # padding (dual-fp8 ldweights wants a pow2 column count)
                    v_sb = qk.tile([128, 8, 2, HPG, 128], FP8, tag="v")
                    nc.vector.memzero(v_sb)
                    nc.gpsimd.dma_start(
                        out=v_sb[:, :, :, :, HD : HD + 1], in_=cpk[:, 0:64]
                    )
                else:
                    # V with ones column per head: [128, jt, head, 65]
                    v_sb = qk.tile([128, 16, HPG, HD + 1], MMT, tag="v")
                    nc.gpsimd.dma_start(
                        out=v_sb[:, :, :, HD : HD + 1], in_=cpk[:, 0:64]
                    )
                wo_sb = const.tile([128, 2, D], MMT, tag="wo")
                # zero-padded hp=1 stationaries for the shift-free tail
                # outproj: [:,0,:] rows 0-63 = Wo hp1/h2=0 rows, [:,1,:]
                # rows 0-63 = Wo hp1/h2=1 rows; rows 64-127 stay zero so
                # K=128 matmuls ignore garbage moving partitions
                woz_sb = const.tile([128, 2, D], MMT, tag="woz")

                nc.sync.dma_start(out=wk_sb[:, :, 128:DG], in_=wk[:, :, 128:DG])
                nc.sync.dma_start(out=wq_sb[:, :, 128:DG], in_=wq[:, :, 128:DG])
                load_xtc(1, nc.gpsimd)
                load_xtc(2, nc.gpsimd)
                load_xtc(3, nc.sync)
                xtc = [[xtc_t[c][:, kt, :] for kt in range(8)] for c in range(4)]

                qt_sb = [
                    qk.tile([128, N], MMT, tag=f"qt{m}", name=f"qt{m}")
                    for m in range(2)
                ]
                kt_sb = [
                    qk.tile([128, N], MMT, tag=f"kt{m}", name=f"kt{m}")
                    for m in range(2)
                ]

                # bv broadcast to all 128 partitions via PE (deferred
                # past the first chains so Pool const DMAs don't gate them)
                bv_bc = const.tile([128, DG], F32, tag="bvbc")

                def bv_bcast():
                    bv_ps = psum_x.tile([128, DG], F32, tag="x", name="bvps")
                    nc.tensor.matmul(
                        bv_ps, ones[0:1, 0:128], bvr_sb[0:1, :],
                        start=True, stop=True,
                    )
                    nc.vector.tensor_copy(bv_bc, bv_ps)

                def qk_chain(proj, hp, nt, defer_bias=False):
                    w_sb, bias_sb, dst = (
                        (wq_sb, bq_sb, qt_sb)
                        if proj == 0
                        else (wk_sb, bk_sb, kt_sb)
                    )
                    ps = psum_x.tile([128, 512], F32, tag="x", name="qkps")
                    for kt in range(8):
                        nc.tensor.matmul(
                            ps,
                            w_sb[:, kt, hp * 128 : (hp + 1) * 128],
                            xtc[nt][kt],
                            start=(kt == 0),
                            stop=(kt == 7),
                        )
                    out_ap = dst[hp][:, nt * 512 : (nt + 1) * 512]

                    def bias():
                        # deferred (boundary-cover) drains are emitted after
                        # norm_b so the DVE norm chain runs first (gpsimd
                        # cannot touch PSUM on real hw)
                        nc.vector.tensor_scalar_add(
                            out_ap, ps, bias_sb[:, hp : hp + 1]
                        )

                    if defer_bias:
                        return bias
                    nc.vector.tensor_scalar_add(
                        out_ap, ps, bias_sb[:, hp : hp + 1]
                    )

                def v_chain(mt):
                    ps = psum_x.tile([128, DG], F32, tag="x", name="vps")
                    for kt in range(8):
                        nc.tensor.matmul(
                            ps,
                            xtc[mt // 4][kt][
                                :, (mt % 4) * 128 : (mt % 4 + 1) * 128
                            ],
                            wv_sb[:, kt, :],
                            start=(kt == 0),
                            stop=(kt == 7),
                        )
                    v_out = (
                        v_sb[:, mt // 2, mt % 2, :, 0:HD]
                        if FP8_CTX
                        else v_sb[:, mt, :, 0:HD]
                    )
                    nc.vector.tensor_tensor(
                        out=v_out,
                        in0=ps.rearrange("p (h d) -> p h d", h=HPG),
                        in1=bv_bc.rearrange("p (h d) -> p h d", h=HPG),
                        op=mybir.AluOpType.add,
                    )

                # ---- unit stream ----
                ctxn = [
                    qk.tile([128, N], MMT, tag=f"ctxn{m}", name=f"ctxn{m}")
                    for m in range(2)
                ]

                blocks = [(q4, hp) for q4 in range(4) for hp in range(2)]
                units = [
                    (b_idx, q4, hp, jt)
                    for b_idx, (q4, hp) in enumerate(blocks)
                    for jt in range(16)
                ]
                ctx_ps_of = {}
                unit_e = {}

                pair_e = {}

                def emit_s_exp(u):
                    b_idx, q4, hp, jt = u
                    s_ps = psum_b.tile([128, 1024], F32, tag="bank", name="sps")
                    for h2 in range(2):
                        nc.tensor.matmul(
                            s_ps[:, h2 * 512 : (h2 + 1) * 512],
                            kt_sb[hp][
                                h2 * 64 : (h2 + 1) * 64,
                                jt * 128 : (jt + 1) * 128,
                            ],
                            qt_sb[hp][
                                h2 * 64 : (h2 + 1) * 64,
                                q4 * 512 : (q4 + 1) * 512,
                            ],
                            start=True,
                            stop=True,
                        )
                    if FP8_CTX:
                        k = jt // 2
                        if jt % 2 == 0:
                            pair_e[(b_idx, k)] = epool.tile(
                                [128, 2, 1024], FP8, tag="e", name="esb"
                            )
                        e_out = pair_e[(b_idx, k)][:, jt % 2, :]
                    else:
                        e_out = epool.tile([128, 1024], MMT, tag="e", name="esb")
                        unit_e[u] = e_out
                    nc.scalar.activation(
                        out=e_out,
                        in_=s_ps,
                        func=mybir.ActivationFunctionType.Exp,
                        bias=mb_sb[:, jt : jt + 1],
                        scale=0.125,
                    )

                def emit_ctx(u):
                    b_idx, q4, hp, jt = u
                    if b_idx not in ctx_ps_of:
                        ctx_ps_of[b_idx] = psum_c.tile(
                            [65, 1024], F32, tag="ctx", name="ctxps"
                        )
                    ctx_ps = ctx_ps_of[b_idx]
                    e_sb = unit_e.pop(u)
                    for h2 in range(2):
                        nc.tensor.matmul(
                            ctx_ps[:, h2 * 512 : (h2 + 1) * 512],
                            v_sb[:, jt, 2 * hp + h2, :],
                            e_sb[:, h2 * 512 : (h2 + 1) * 512],
                            start=(jt == 0),
                            stop=(jt == 15),
                            skip_group_check=True,
                        )

                def emit_ctx_pair(k):
                    b_idx = k // 8
                    jp = k % 8
                    q4, hp = blocks[b_idx]
                    if b_idx not in ctx_ps_of:
                        ctx_ps_of[b_idx] = psum_c.tile(
                            [128, 1024], F32, tag="ctx", name="ctxps"
                        )
                    ctx_ps = ctx_ps_of[b_idx]
                    e8 = pair_e.pop((b_idx, jp))
                    for h2 in range(2):
                        nc.tensor.matmul(
                            ctx_ps[:, h2 * 512 : (h2 + 1) * 512],
                            v_sb[:, jp, :, 2 * hp + h2, :],
                            e8[:, :, h2 * 512 : (h2 + 1) * 512],
                            start=(jp == 0),
                            stop=(jp == 7),
                            perf_mode=mybir.MatmulPerfMode.DoubleRow,
                            skip_group_check=True,
                        )

                norm_r = {}

                def emit_norm_a(b_idx):
                    # reciprocal of the denominators (row 64) -- issued to
                    # DVE right after the block's last ctx matmul so it runs
                    # while the PE chews boundary-cover work
                    ctx_ps = ctx_ps_of[b_idx]
                    r_sb = rpool.tile([65, 1024], MMT, tag="r", name="rsb")
                    for h2 in (1, 0):
                        nc.vector.reciprocal(
                            out=r_sb[64:65, h2 * 512 : (h2 + 1) * 512],
                            in_=ctx_ps[64:65, h2 * 512 : (h2 + 1) * 512],
                        )
                    norm_r[b_idx] = r_sb

                tail_tmp = {}

                def emit_norm_b(b_idx, copy_eng=None, skip_shift=False):
                    q4, hp = blocks[b_idx]
                    ctx_ps = ctx_ps_of.pop(b_idx)
                    r_sb = norm_r.pop(b_idx)
                    # broadcast matmuls into one [64,1024] tile (a matmul
                    # output may not cross a PSUM bank: 512 f32 per half)
                    rp = psum_b.tile([64, 1024], F32, tag="bank", name="rp")
                    for h2 in (1, 0):
                        nc.tensor.matmul(
                            rp[:, h2 * 512 : (h2 + 1) * 512],
                            ones[64:65, 0:64],
                            r_sb[64:65, h2 * 512 : (h2 + 1) * 512],
                            start=True,
                            stop=True,
                            tile_position=(64, 0),
                        )
                    # rp must round-trip through SBUF (DVE may read only
                    # one PSUM input); ACT is the bottleneck engine so the
                    # copies ride DVE.
                    for h2 in (1, 0):
                        (copy_eng or nc.vector.tensor_copy)(
                            r_sb[0:64, h2 * 512 : (h2 + 1) * 512],
                            rp[:, h2 * 512 : (h2 + 1) * 512],
                        )
                    # h2=1 first so its partition-shift overlaps the h2=0
                    # multiply
                    tmp_full = rpool.tile(
                        [128, 512], MMT, tag="tmp", name="tmp"
                    )
                    tmp = tmp_full[0:64, :]
                    nc.vector.tensor_tensor(
                        out=tmp,
                        in0=ctx_ps[0:64, 512:1024],
                        in1=r_sb[0:64, 512:1024],
                        op=mybir.AluOpType.mult,
                    )
                    if skip_shift:
                        # the tail outproj reads tmp directly (3-matmul
                        # form) instead of paying the ~2.2us shift-DMA
                        # latency on the critical path
                        nc.vector.memzero(tmp_full[64:128, :])
                        tail_tmp[b_idx] = tmp_full
                    else:
                        nc.sync.dma_start(
                            out=ctxn[hp][64:128, q4 * 512 : (q4 + 1) * 512],
                            in_=tmp,
                        )
                    nc.vector.tensor_tensor(
                        out=ctxn[hp][0:64, q4 * 512 : (q4 + 1) * 512],
                        in0=ctx_ps[0:64, 0:512],
                        in1=r_sb[0:64, 0:512],
                        op=mybir.AluOpType.mult,
                    )

                def emit_outproj(q4, mo, copy_eng=None, defer_copy=False):
                    ps = psum_x.tile([128, 512], F32, tag="x", name="ops")
                    for hp in range(2):
                        nc.tensor.matmul(
                            ps,
                            wo_sb[:, hp, mo * 128 : (mo + 1) * 128],
                            ctxn[hp][:, q4 * 512 : (q4 + 1) * 512],
                            start=(hp == 0),
                            stop=(hp == 1),
                        )

                    def drain(eng=None):
                        ob = big.tile([128, 512], F32, tag="ob", name="ob")
                        (eng or copy_eng or nc.vector.tensor_copy)(ob, ps)
                        dq = nc.gpsimd if (q4 == 3 and mo % 2 == 1) else nc.sync
                        dq.dma_start(
                            out=outp[
                                mo * 128 : (mo + 1) * 128,
                                q4 * 512 : (q4 + 1) * 512,
                            ],
                            in_=ob,
                        )

                    if defer_copy:
                        return drain
                    drain()

                # ---- schedule ----
                # sched[i]: callables emitted after S(i) and ctx(i-2), in
                # order. Block boundaries (i = 16b+17): norm_a (recip on
                # DVE), cover matmuls, norm_b (broadcast + mults), then the
                # covers' deferred DVE bias/copies -- so the norm's DVE chain
                # is never queued behind cover consumers, and cover psum
                # consumers don't stall the next psum_x allocations.
                sched = {}

                def at(i, fn):
                    sched.setdefault(i, []).append(fn)

                def ch(i, proj, hp, nt):
                    at(i, lambda: qk_chain(proj, hp, nt))

                def op(i, q4, mo, eng=None):
                    at(i, lambda: emit_outproj(q4, mo, eng))

                def boundary(i, b, cover1, cover2):
                    def emit():
                        emit_norm_a(b)
                        drains = [fn() for fn in cover1]
                        emit_norm_b(b)
                        for d in drains:
                            d()

                    at(i, emit)
                    # cover2 lands one unit later so S(i+1) (and its exp)
                    # isn't queued behind the whole boundary burst
                    for fn in cover2:
                        at(i + 1, fn)

                def dch(proj, hp, nt):
                    return lambda: qk_chain(proj, hp, nt, defer_bias=True)

                def dop(q4, mo):
                    return lambda: emit_outproj(q4, mo, defer_copy=True)

                at(0, bv_bcast)
                at(0, lambda: v_chain(0))
                for j in range(1, 16):
                    at(j, lambda m=j: v_chain(m))
                ch(3, 1, 0, 1)
                ch(6, 1, 0, 2)
                ch(9, 1, 0, 3)
                ch(11, 1, 1, 0)
                ch(13, 1, 1, 1)
                ch(14, 0, 1, 0)
                boundary(17, 0, [dch(1, 1, 2)], [lambda: qk_chain(1, 1, 3)])
                at(20, lambda: nc.gpsimd.dma_start(out=wo_sb, in_=wo[:, :, :]))

                def load_woz():
                    nc.vector.memzero(woz_sb)
                    nc.gpsimd.dma_start(
                        out=woz_sb[0:64, 0, :], in_=wo[0:64, 1, :]
                    )
                    nc.gpsimd.dma_start(
                        out=woz_sb[0:64, 1, :], in_=wo[64:128, 1, :]
                    )
                    # tail-only: ctxn[1] partitions 64-127 at q4=3 are never
                    # written in skip-shift mode; zero them so garbage*0
                    # cannot produce NaN in the K=128 tail matmuls
                    nc.vector.memzero(ctxn[1][64:128, 1536:2048])

                at(22, load_woz)
                ch(28, 0, 0, 1)
                boundary(33, 1, [dch(0, 1, 1)], [lambda: qk_chain(0, 0, 2)])
                op(36, 0, 0)
                op(38, 0, 1)
                boundary(
                    49, 2, [dch(0, 1, 2)],
                    [lambda: emit_outproj(0, 2), lambda: emit_outproj(0, 3)],
                )
                op(52, 0, 4)
                op(54, 0, 5)
                boundary(
                    65, 3, [dch(0, 0, 3)],
                    [lambda: emit_outproj(0, 6), lambda: emit_outproj(0, 7)],
                )
                boundary(
                    81, 4, [dch(0, 1, 3)],
                    [lambda: emit_outproj(1, 0), lambda: emit_outproj(1, 1)],
                )
                op(84, 1, 2)
                op(86, 1, 3)
                boundary(
                    97, 5, [dop(1, 4), dop(1, 5)],
                    [lambda: emit_outproj(1, 6), lambda: emit_outproj(1, 7)],
                )
                boundary(
                    113, 6, [dop(2, 0), dop(2, 1)],
                    [lambda: emit_outproj(2, 2), lambda: emit_outproj(2, 3)],
                )
                op(116, 2, 4)
                op(118, 2, 5)
                op(120, 2, 6)
                op(122, 2, 7)

                # prologue chains
                qk_chain(1, 0, 0)
                qk_chain(0, 0, 0)

                for i, u in enumerate(units):
                    emit_s_exp(u)
                    if FP8_CTX:
                        if i >= 3 and i % 2 == 1:
                            emit_ctx_pair((i - 3) // 2)
                    elif i >= 2:
                        emit_ctx(units[i - 2])
                    for fn in sched.get(i, []):
                        fn()
                if FP8_CTX:
                    emit_ctx_pair(63)
                else:
                    emit_ctx(units[-2])
                    emit_ctx(units[-1])
                emit_norm_a(7)
                emit_norm_b(7, copy_eng=nc.scalar.copy, skip_shift=True)
                tmp7 = tail_tmp.pop(7)
                for mo in range(8):
                    pool = psum_x if mo % 2 == 0 else psum_b
                    tag = "x" if mo % 2 == 0 else "bank"
                    ps = pool.tile([128, 512], F32, tag=tag, name="ops")
                    nc.tensor.matmul(
                        ps,
                        wo_sb[:, 0, mo * 128 : (mo + 1) * 128],
                        ctxn[0][:, 1536:2048],
                        start=True,
                        stop=False,
                        skip_group_check=True,
                    )
                    nc.tensor.matmul(
                        ps,
                        woz_sb[:, 0, mo * 128 : (mo + 1) * 128],
                        ctxn[1][:, 1536:2048],
                        start=False,
                        stop=False,
                        skip_group_check=True,
                    )
                    nc.tensor.matmul(
                        ps,
                        woz_sb[:, 1, mo * 128 : (mo + 1) * 128],
                        tmp7,
                        start=False,
                        stop=True,
                        skip_group_check=True,
                    )
                    ob = big.tile([128, 512], F32, tag="ob", name="ob")
                    (nc.scalar.copy if mo % 2 == 0 else nc.vector.tensor_copy)(
                        ob, ps
                    )
                    dq = (nc.sync, nc.gpsimd, nc.scalar)[mo % 3]
                    dq.dma_start(
                        out=outp[mo * 128 : (mo + 1) * 128, 1536:2048],
                        in_=ob,
                    )


    nc.finalize()
    return nc


_NC_CACHE = None


def _get_program():
    global _NC_CACHE
    if _NC_CACHE is None:
        _NC_CACHE = build_program()
    return _NC_CACHE


def make_in_maps(X, mask, Wq, bq, Wk, bk, Wv, bv, Wo, bo):
    X = np.asarray(X, dtype=np.float32)
    mask = np.asarray(mask, dtype=np.float32)
    in_maps = []
    xts = [np.ascontiguousarray(X[b].T) for b in range(B)]
    mbs = [
        np.ascontiguousarray((-1e6 * (1.0 - mask[b])).reshape(16, 128).T)
        for b in range(B)
    ]
    for c in range(8):
        b, g = c // HG, c % HG
        sl = slice(g * DG, (g + 1) * DG)
        wq_s = np.ascontiguousarray(
            np.asarray(Wq[:, sl]).reshape(8, 128, DG).transpose(1, 0, 2)
        )
        wk_s = np.ascontiguousarray(
            np.asarray(Wk[:, sl]).reshape(8, 128, DG).transpose(1, 0, 2)
        )
        wv_s = np.ascontiguousarray(
            np.asarray(Wv[:, sl]).reshape(8, 128, DG).transpose(1, 0, 2)
        )
        bq_s = np.ascontiguousarray(np.asarray(bq[sl]).reshape(2, 128).T)
        bk_s = np.ascontiguousarray(np.asarray(bk[sl]).reshape(2, 128).T)
        bv_s = np.ascontiguousarray(np.asarray(bv[sl]).reshape(1, DG))
        # Wo rows for this group, pair-packed: [64*h2+p, kt, o] = Wo[g*256+(2kt+h2)*64+p, o]
        wo_s = np.ascontiguousarray(
            np.asarray(Wo[sl, :]).reshape(2, 2, 64, D).transpose(1, 2, 0, 3)
            .reshape(128, 2, D)
        )
        cpk = np.zeros((128, 212), dtype=np.float32)
        cpk[:, 0:128] = 1.0
        cpk[:, 128:130] = bq_s
        cpk[:, 130:132] = bk_s
        cpk[:, 132:148] = mbs[b]
        cpk[0:64, 148:212] = np.eye(64, dtype=np.float32)
        in_maps.append(
            {
                "xt": xts[b],
                "cpk": cpk,
                "wq": wq_s.astype(np.float32),
                "wk": wk_s.astype(np.float32),
                "wv": wv_s.astype(np.float32),
                "bvr": bv_s.astype(np.float32),
                "wo": wo_s.astype(np.float32),
            }
        )
    return in_maps


def gather_output(results, bo):
    out = np.zeros((B, N, D), dtype=np.float32)
    for c in range(8):
        out[c // HG] += results[c]["outp"].T
    out += np.asarray(bo, dtype=np.float32)
    return out


def kernel(**inputs):
    from concourse import bass_utils

    nc = _get_program()
    in_maps = make_in_maps(**inputs)
    res = bass_utils.run_bass_kernel_spmd(nc, in_maps, core_ids=list(range(8)))
    return gather_output(res.results, inputs["bo"])
